# revision 82
# baseline (speedup 1.0000x reference)
"""AttentionBlock Trainium2 Bass kernel.

Problem: x[16,512,32,32] -> qkv proj -> 8-head attention (dk=64) over the
1024 spatial positions -> out proj + residual -> [16,512,32,32].

Sharding: data-parallel over batch; 2 images per core on 8 cores.

All compute happens in "transposed" (feature-major) space, which is the
natural layout of the inputs -- x arrives as [C, H*W] per image -- so the
kernel needs zero on-chip transposes:
  qT,kT  : [dk, tok]  = w_q_cols.T @ x        (fp8 DoubleRow, channel pairs)
  v      : [tok, dk]  = x_tile.T @ w_v_cols   (fp8 DoubleRow)
  S^T    : [j, i]     = kT_slice.T @ qT       (bf16, K = dk = 64)
  P^T    : exp(S^T*scale - 2) on ScalarE -> fp8e4 directly
  res^T  : [dk+1, i]  = v_aug.T @ P^T         (fp8 DoubleRow over j pairs;
                                               "ones"=16 col -> denom row)
  out^T  : [c, t]     = w_out_rows.T @ res^T  (bf16, split A/B, see below)

fp8 scaling: w_q/w_k/w_v are host-scaled x16 so their N(0, 1/sqrt(C))
entries clear the e4m3 subnormal range; exp rescales by scale/256 (q and k
both carry x16).  The v tiles are 16*v, the ones column is 16.0, so
r = (16*num)/(16*den) = res exactly; w_out needs no rescale.  exp gets a
-2 bias (cancels in the softmax ratio) so P stays under e4m3's 240 max
(float8e4 here is e4m3 WITH inf: max finite 240).  Measured end-to-end rel
err ~5e-3 (gate 2e-2).

Softmax normalization: reciprocal of the denominator row (DVE), broadcast
across partitions on the otherwise-idle GPSIMD engine, one DVE multiply.
Heads are stored pairwise in 128-partition tiles; odd heads' normalized
results are partition-shifted into rows 64:128 with an SBUF->SBUF DMA.
GPSIMD cannot touch PSUM, so every PSUM read/copy sits on DVE or ACT.

Output projection is split into two half-contractions so the post-last-exp
drain is short: the A-half (pairs 0-2 + the host-folded bf16 residual
x + b_out, added via an identity matmul) accumulates mid-image as soon as
those pairs normalize, and only the B-half (pair 3 + the A result re-added
via an fp32r identity matmul) remains after the final exp.  y leaves as
bf16, one descriptor per 128-channel chunk.

Schedule: ScalarE (exp) is the bottleneck engine (~133us of the ~160us
span), so everything else hides under it:
 - startup runs only pair-0 qkv + v(g0) before the first exp; the rest of
   image 0's projections stream in as fill during its own attention
 - dummy matmuls keep the PE busy through the startup DMAs so the first
   real matmuls run at full HAM clock
 - each head's last AV + normalization are deferred past the next head's
   first score matmul, so the next exp is never blocked by them
 - a warm-up exp at t=0 pre-loads the ACT function table during the DMAs
 - five exp tiles (last image's early heads, where DVE has slack) run as
   Schraudolph fast-exp on the DVE: one int16 mult-add writes bf16 bits
   (~1.6% rel err, hidden under fp8 P quantization) and the deferred AV
   consumes the bitcast directly through mixed fp8(v) x bf16(P) matmuls,
   shaving those slots off the ScalarE bottleneck; their AV groups defer
   two extra ticks so the DVE path never blocks the in-order PE queue
 - attention (exp-bound) of image i overlaps projections of image i+1 and
   the output projections of images i-1/i.
"""

from contextlib import ExitStack

import ml_dtypes
import numpy as np

import concourse.bass as bass
import concourse.mybir as mybir
import concourse.tile as tile
from concourse import bacc
from concourse.bass_utils import run_bass_kernel_spmd

F32 = mybir.dt.float32
F32R = mybir.dt.float32r
I16 = mybir.dt.int16
BF16 = mybir.dt.bfloat16
FP8 = mybir.dt.float8e4   # ml_dtypes.float8_e4m3: max finite 240
VSTR = 80                 # per-head stride in the fp8 v tile (16B-aligned)
ONES = 16.0               # denominator column value (cancels the x16 v scale)
WSC = 16.0                # host-side fp8 weight prescale
EXP_BIAS = -2.0           # exp(S*scale - 2): keeps P in fp8 range; cancels
                          # in the softmax ratio

N_CORES = 8
B_LOC = 2            # images per core
C = 512              # channels
NTOK = 1024          # 32*32 spatial positions
NH = 8               # heads
DK = 64              # head dim
NPAIR = 4            # head pairs
CCH = 4              # channel chunks of 128
CCP = 2              # channel chunk pairs (DoubleRow k-tiles)
TCH = 2              # token chunks of 512
SCALE = DK ** -0.5
MODE = "fp8"
# Schraudolph fast-exp on DVE for offloaded tiles: exp(x) ~=
# bf16_bits_as_float(int16(x * 2^7/ln2 + (127<<7) - 6)); ~1.6% mean rel
# err, hidden under the existing fp8 P quantization.  x = sps*k + EXP_BIAS
# folds into the two tensor_scalar constants; the AV matmul reads the
# int16 tile bitcast to bf16 (mixed fp8 x bf16 operands are PE-legal).
SCH_A = float(1 << 7) / float(np.log(2.0))   # bf16-bit scale
SCH_C1 = (SCALE / (WSC * WSC)) * SCH_A
SCH_C2 = float(127 << 7) - 6.0 + EXP_BIAS * SCH_A

DR = mybir.MatmulPerfMode.DoubleRow


def _emit(tc, x_d, xb_d, wq_d, wk_d, wv_d, wo_d, bqp_d, bkp_d, id_d, idb_d,
          bv_d, y_d, repeat=1):
    nc = tc.nc
    ADD = mybir.AluOpType.add

    with ExitStack() as ctx:
        cst = ctx.enter_context(tc.tile_pool(name="cst", bufs=1))
        w8_p = ctx.enter_context(tc.tile_pool(name="w8", bufs=3 * CCP))
        wo_p = ctx.enter_context(tc.tile_pool(name="wo", bufs=NPAIR))
        x_p = ctx.enter_context(tc.tile_pool(name="xp", bufs=2 * CCH))
        xb_p = ctx.enter_context(tc.tile_pool(name="xbp", bufs=2 * CCP + 1))
        q_p = ctx.enter_context(tc.tile_pool(name="qp", bufs=2 * NPAIR))
        k_p = ctx.enter_context(tc.tile_pool(name="kp", bufs=2 * NPAIR))
        v_p = ctx.enter_context(tc.tile_pool(name="vp", bufs=10))
        pt_p = ctx.enter_context(tc.tile_pool(name="ptp", bufs=8))
        r_p = ctx.enter_context(tc.tile_pool(name="rp", bufs=2 * NPAIR))
        t_p = ctx.enter_context(tc.tile_pool(name="tp", bufs=3))
        dn_p = ctx.enter_context(tc.tile_pool(name="dnp", bufs=3))
        rs_p = ctx.enter_context(tc.tile_pool(name="rsp", bufs=4))
        bc_p = ctx.enter_context(tc.tile_pool(name="bcp", bufs=4))
        o_p = ctx.enter_context(tc.tile_pool(name="op", bufs=4))
        oa_p = ctx.enter_context(tc.tile_pool(name="oap", bufs=10))
        si_p = ctx.enter_context(tc.tile_pool(name="sip", bufs=2))
        ps = ctx.enter_context(tc.tile_pool(name="ps", bufs=4, space="PSUM"))
        ps_big = ctx.enter_context(tc.tile_pool(name="psb", bufs=2, space="PSUM"))

        # ---- PE p-state warm-up ----------------------------------------
        # dummy matmuls keep the tensor engine continuously busy through
        # the startup DMA window, so the first real matmuls run at full
        # clock instead of paying the cold HAM ramp
        def em_warm_pe(n=4):
            dl = cst.tile([128, 2], BF16, tag="wl", name="warm_l")
            dr = cst.tile([128, 512], BF16, tag="wr", name="warm_r")
            nc.vector.memset(dl[:], 0.0)
            nc.vector.memset(dr[:], 0.0)
            wp = ps.tile([2, 512], F32, tag="ps", name="warm_ps")
            for n_ in range(n):
                nc.tensor.matmul(wp[:], dl[:], dr[:],
                                 start=(n_ == 0), stop=(n_ == n - 1))

        # ---- constants / biases (host-prepped, single DMAs) ------------
        def em_bqp():
            # bqp rides directly behind xb8_0: the single HWDGE unit
            # serializes every sync/ACT descriptor at ~650ns, so ordering
            # on that stream IS the startup critical path
            bqp = cst.tile([128, NPAIR], F32, tag="bqp", name="bqp")
            nc.sync.dma_start(out=bqp[:], in_=bqp_d)
            return bqp

        def em_biases():
            # exp bias + warm-up exp first: the table load must clear the
            # ACT queue before the startup k copies land on it
            eb = cst.tile([128, 1], F32, tag="eb", name="exp_bias")
            nc.gpsimd.memset(eb[:], EXP_BIAS)
            warm = cst.tile([128, 1], F32, tag="warm", name="act_warm")
            nc.scalar.activation(
                warm[:], eb[:], mybir.ActivationFunctionType.Exp)
            bkp = cst.tile([128, NPAIR], F32, tag="bkp", name="bkp")
            nc.sync.dma_start(out=bkp[:], in_=bkp_d)
            bv_st = cst.tile([1, NH * DK], F32, tag="bvst", name="bv_st")
            nc.gpsimd.dma_start(out=bv_st[:], in_=bv_d)
            bv_bc = cst.tile([128, NH * DK], F32, tag="bvbc", name="bv_bc")
            nc.gpsimd.partition_broadcast(bv_bc[:], bv_st[:])
            return bkp, bv_bc, eb

        def em_idents():
            # identity operands are first read ~60us in (outproj A);
            # keep their descriptors off the startup-critical HWDGE stream
            idt = cst.tile([128, 128], F32R, tag="idt", name="ident")
            nc.sync.dma_start(out=idt[:], in_=id_d)
            idb = cst.tile([128, 128], BF16, tag="idb", name="identb")
            nc.sync.dma_start(out=idb[:], in_=idb_d)
            return idt, idb

        # ---- weights (host-prepped x16 fp8, DoubleRow layout) ----------
        w_kind = {"q": [], "k": [], "v": []}
        wo_t = []
        W_DRAM = {"q": wq_d, "k": wk_d, "v": wv_d}

        def em_weights(kind, engine=None):
            eng = engine or nc.sync
            for cp in range(CCP):
                wt = w8_p.tile([128, CCP, NH * DK], FP8, tag=f"w{kind}",
                               name=f"w{kind}{cp}", bufs=CCP)
                eng.dma_start(out=wt[:], in_=W_DRAM[kind][cp])
                w_kind[kind].append(wt)

        def em_wout():
            for m in range(NPAIR):
                wt = wo_p.tile([128, C], BF16, tag="wot", name=f"wot{m}")
                nc.sync.dma_start(out=wt[:],
                                    in_=wo_d[m * 128:(m + 1) * 128, :])
                wo_t.append(wt)

        # ---- per-image stages ------------------------------------------
        n_imgs = B_LOC * repeat
        state = {}

        def em_x(i):
            b = i % B_LOC
            xb_t = []
            for cp in range(CCP):
                xb = xb_p.tile([128, CCP, NTOK], FP8, tag="xbt",
                               name=f"xb{i}_{cp}")
                nc.sync.dma_start(out=xb[:], in_=xb_d[b, cp])
                xb_t.append(xb)
            state[i] = {"x": [], "xb": xb_t, "q": {}, "k": {}, "v": {},
                        "r": None, "oa": {}, "otw": {}}

        def em_xres(i):
            # residual fp32 x: only needed at outproj time; keep it off the
            # startup-critical HWDGE queue
            b = i % B_LOC
            for cc in range(CCH):
                xt = x_p.tile([128, NTOK], BF16, tag="xt", name=f"xt{i}_{cc}")
                nc.gpsimd.dma_start(out=xt[:],
                                    in_=x_d[b, cc * 128:(cc + 1) * 128, :])
                state[i]["x"].append(xt)

        def em_qkv_unit(i, m, kind, ch, eng=None):
            # one PSUM group: quarter of a head-pair projection, two fp8
            # DoubleRow matmuls (channel-chunk pairs as k-tiles)
            st = state[i]
            pool, bcol = (q_p, bqp) if kind == "q" else (k_p, bkp)
            if m not in st[kind]:
                st[kind][m] = pool.tile([128, NTOK], BF16, tag=f"{kind}t",
                                        name=f"{kind}t{i}_{m}")
            dst = st[kind][m]
            ps_qk = ps.tile([128, 512], F32, tag="ps",
                            name=f"psqk{i}_{m}_{kind}_{ch}")
            for cp in range(CCP):
                nc.tensor.matmul(
                    ps_qk[:],
                    w_kind[kind][cp][:, :, m * 128:(m + 1) * 128],
                    st["xb"][cp][:, :, ch * 512:(ch + 1) * 512],
                    start=(cp == 0), stop=(cp == CCP - 1), perf_mode=DR)
            # copy out + per-partition bias.  GPSIMD cannot read PSUM, so
            # the startup pair uses ACT (Identity with a bias AP, same
            # table set as Exp) to halve the serial copy chain instead.
            if eng is nc.scalar:
                nc.scalar.activation(
                    dst[:, ch * 512:(ch + 1) * 512], ps_qk[:],
                    mybir.ActivationFunctionType.Identity,
                    bias=bcol[:, m:m + 1])
            else:
                nc.vector.tensor_scalar_add(
                    dst[:, ch * 512:(ch + 1) * 512], ps_qk[:],
                    bcol[:, m:m + 1])

        def em_qkv_pair(i, m):
            for kind in ("q", "k"):
                for ch in range(TCH):
                    em_qkv_unit(i, m, kind, ch)

        def em_v(i, tt):
            # fp8 v tiles per j-group g = tt//2: [128, 2, NH*VSTR] with the
            # two 128-token chunks as AV DoubleRow k-tiles; per head: 64
            # v dims, a 16.0 column (-> x16 denominator row), padding
            st = state[i]
            g, t = divmod(tt, 2)
            ps_v = ps.tile([128, 512], F32, tag="ps", name=f"psv{i}_{tt}")
            for cp in range(CCP):
                nc.tensor.matmul(
                    ps_v[:],
                    st["xb"][cp][:, :, tt * 128:(tt + 1) * 128],
                    w_kind["v"][cp][:],
                    start=(cp == 0), stop=(cp == CCP - 1), perf_mode=DR)
            if g not in st["v"]:
                st["v"][g] = v_p.tile([128, 2, NH * VSTR], FP8, tag="vt",
                                      name=f"vt{i}_{g}")
            vv = st["v"][g][:, t, :].rearrange("p (h e) -> p h e", h=NH)
            nc.vector.memset(vv[:, :, DK:DK + 1], ONES)
            # v bias varies along the free dim: add pre-broadcast rows
            nc.vector.tensor_tensor(
                vv[:, :, 0:DK],
                ps_v.rearrange("p (h d) -> p h d", h=NH),
                bv_bc[:].rearrange("p (h d) -> p h d", h=NH),
                op=ADD)

        # deferred tail of each head: last AV group + normalization; runs
        # right after the NEXT head's first score matmul so the next exp is
        # never stuck behind it
        pend = [None, False]

        def flush_pend():
            if pend[0] is not None:
                fn, pend[0] = pend[0], None
                fn()

        def em_norm(i, h, m, hf, res_ps, last=False):
            st = state[i]
            # copy res to SBUF first (releases the PSUM slots quickly),
            # then reciprocal of the denom row, GPSIMD partition-broadcast,
            # one DVE multiply.  For the final head the copy is skipped
            # (nobody needs those PSUM slots again) to shorten the tail.
            tmp = (t_p.tile([64, NTOK], BF16, tag="tmp", name=f"tmp{i}_{h}")
                   if hf == 1 else None)
            for ic in range(TCH):
                if last:
                    rsb = res_ps[ic]
                else:
                    rsb = rs_p.tile([DK + 1, 512], F32, tag="rsb",
                                    name=f"rsb{i}_{h}_{ic}")
                    nc.vector.tensor_copy(rsb[:], res_ps[ic][0:DK + 1, :])
                rc = dn_p.tile([1, 512], F32, tag="dn", name=f"rc{i}_{h}_{ic}")
                nc.vector.reciprocal(rc[:], rsb[DK:DK + 1, :])
                bc = bc_p.tile([64, 512], F32, tag="bc",
                               name=f"bc{i}_{h}_{ic}")
                nc.gpsimd.partition_broadcast(bc[:], rc[:])
                dst = (st["r"][m] if hf == 0 else tmp)[0:DK,
                                                       ic * 512:(ic + 1) * 512]
                nc.vector.tensor_tensor(
                    dst, rsb[0:DK, :], bc[:],
                    op=mybir.AluOpType.mult)
            if hf == 1:
                # partition-shift odd head into rows 64:128 of pair tile
                nc.gpsimd.dma_start(out=st["r"][m][64:128, :],
                                    in_=tmp[0:DK, :])

        def em_attn_head(i, h, last=False, sch=False):
            st = state[i]
            if st["r"] is None:
                st["r"] = [r_p.tile([128, NTOK], BF16, tag="rt",
                                    name=f"rt{i}_{n}") for n in range(NPAIR)]
            m, hf = divmod(h, 2)
            rs = slice(64 * hf, 64 * hf + 64)
            res_ps = None
            pt_g = None
            sch_it = None
            for j in range(NH):
                yield
                sps = ps_big.tile([128, NTOK], F32, tag="sps",
                                  name=f"sps{i}_{h}_{j}")
                for ic in range(TCH):
                    nc.tensor.matmul(
                        sps[:, ic * 512:(ic + 1) * 512],
                        st["k"][m][rs, j * 128:(j + 1) * 128],
                        st["q"][m][rs, ic * 512:(ic + 1) * 512],
                        start=True, stop=True)
                if j == 0:
                    # previous head's deferred AV+norm go after our first
                    # score matmul; only then reuse its PSUM slots.  If the
                    # previous head's last exp ran on the DVE, its pt lands
                    # late -- push the flush two ticks further so the AV
                    # never blocks our score matmuls in the in-order PE queue
                    if not pend[1]:
                        flush_pend()
                    res_ps = [ps.tile([128, 512], F32, tag="ps",
                                      name=f"res{i}_{h}_{n}")
                              for n in range(TCH)]
                elif j == 3 and pend[1]:
                    # after this tick's score matmul: the AV burst of the
                    # previous (offloaded) head then lands behind S j3 in
                    # the PE queue instead of in front of it
                    flush_pend()
                    pend[1] = False
                g, t = divmod(j, 2)
                if t == 0:
                    pt_g = pt_p.tile([128, 2, NTOK], FP8, tag="pt",
                                     name=f"pt{i}_{h}_{g}")
                if sch and j == NH - 1:
                    # Schraudolph fast-exp on DVE, writing bf16 bits via an
                    # int16 mult-add: one DVE pass, no fp8 re-cast -- the
                    # deferred AV consumes the bitcast directly through a
                    # mixed fp8(v) x bf16(P) matmul
                    sch_it = si_p.tile([128, NTOK], I16, tag="si",
                                       name=f"sch{i}_{h}")
                    nc.vector.tensor_scalar(
                        sch_it[:], sps[:], SCH_C1, SCH_C2,
                        op0=mybir.AluOpType.mult, op1=ADD)
                else:
                    nc.scalar.activation(
                        pt_g[:, t, :], sps[:],
                        mybir.ActivationFunctionType.Exp,
                        scale=SCALE / (WSC * WSC), bias=ebias[:])
                if t == 1:
                    # fp8 DoubleRow AV: both 128-token k-tiles of the group
                    # in one matmul at 0.5 cycles/row.  For Schraudolph
                    # heads the last group splits into two single-k-tile
                    # matmuls: t0 plain fp8, t1 mixed fp8(v) x bf16(P)
                    def em_av(g=g, pt_l=pt_g):
                        for ic in range(TCH):
                            nc.tensor.matmul(
                                res_ps[ic][0:DK + 1, :],
                                st["v"][g][:, :, h * VSTR:h * VSTR + DK + 1],
                                pt_l[:, :, ic * 512:(ic + 1) * 512],
                                start=(g == 0), stop=(g == NH // 2 - 1),
                                perf_mode=DR)

                    def em_av_sch(g=g, pt_l=pt_g, it=sch_it):
                        pb = it[:].bitcast(BF16)
                        for ic in range(TCH):
                            nc.tensor.matmul(
                                res_ps[ic][0:DK + 1, :],
                                st["v"][g][:, 0,
                                           h * VSTR:h * VSTR + DK + 1],
                                pt_l[:, 0, ic * 512:(ic + 1) * 512],
                                start=False, stop=False)
                            nc.tensor.matmul(
                                res_ps[ic][0:DK + 1, :],
                                st["v"][g][:, 1,
                                           h * VSTR:h * VSTR + DK + 1],
                                pb[:, ic * 512:(ic + 1) * 512],
                                start=False, stop=True)
                    if g < NH // 2 - 1:
                        em_av()
                    else:
                        def fl(res_ps=res_ps, use_sch=sch):
                            (em_av_sch if use_sch else em_av)()
                            em_norm(i, h, m, hf, res_ps, last=last)
                        pend[0] = fl
                        pend[1] = sch

        # output projection, split in two half-contractions: the pairs-0/1
        # half (A) runs as soon as those pairs normalize (mid-image), so the
        # post-last-exp tail only carries the pairs-2/3 half (B) + combine
        def em_outproj_a(i, idx, part, spill=False):
            # pairs 0-2 of the contraction + the residual (x + b_out, host-
            # folded, bf16) added via an identity matmul -- the A-half
            # lands in one PSUM group, one DVE copy out.  Emitted in two
            # parts so the PE bursts stay short between score matmuls.
            st = state[i]
            ct, ch = divmod(idx, TCH)
            if part == 0:
                opsA = ps.tile([128, 512], F32, tag="ps",
                               name=f"psoa{i}_{idx}")
                st["oaps"] = opsA
                nc.tensor.matmul(
                    opsA[:], identb[:],
                    st["x"][ct][:, ch * 512:(ch + 1) * 512],
                    start=True, stop=False)
                nc.tensor.matmul(
                    opsA[:],
                    wo_t[0][:, ct * 128:(ct + 1) * 128],
                    st["r"][0][:, ch * 512:(ch + 1) * 512],
                    start=False, stop=False)
                return
            opsA = st["oaps"]
            for m in (1, 2):
                nc.tensor.matmul(
                    opsA[:],
                    wo_t[m][:, ct * 128:(ct + 1) * 128],
                    st["r"][m][:, ch * 512:(ch + 1) * 512],
                    start=False, stop=(m == 2))
            oa = oa_p.tile([128, 512], F32R, tag="oa", name=f"oa{i}_{idx}")
            if spill:
                # post-loop spill: DVE is about to run the normalization
                # chain -- use the idle ACT engine for this copy instead
                nc.scalar.activation(oa[:], opsA[:],
                                     mybir.ActivationFunctionType.Copy)
            else:
                nc.vector.tensor_copy(oa[:], opsA[:])
            st["oa"][idx] = oa

        def em_b_open(i, idx, opsB):
            # B-half group opens with the A-result added back via an fp32r
            # identity matmul: runs as soon as the PSUM slot frees, while
            # pair 3 is still normalizing
            nc.tensor.matmul(
                opsB, ident[:], state[i]["oa"][idx][:],
                start=True, stop=False)

        def em_b_close(i, idx, opsB, tail=False):
            b = i % B_LOC
            st = state[i]
            ct, ch = divmod(idx, TCH)
            nc.tensor.matmul(
                opsB,
                wo_t[3][:, ct * 128:(ct + 1) * 128],
                st["r"][3][:, ch * 512:(ch + 1) * 512],
                start=False, stop=True)
            # both token halves of a ct-chunk share one wide SBUF tile so
            # each y write is a single descriptor (HWDGE descriptor gen at
            # ~625ns each is the drain's DMA bottleneck, not bandwidth)
            otw = st["otw"]
            if ct not in otw:
                otw[ct] = o_p.tile([128, NTOK], BF16, tag="ot",
                                   name=f"ot{i}_{ct}")
            dst = otw[ct][:, ch * 512:(ch + 1) * 512]
            # GPSIMD cannot read PSUM: copies go DVE mid-stream, and
            # alternate DVE / ACT in the drain (ACT is free after the
            # last exp)
            if tail and idx % 2 == 1:
                nc.scalar.activation(dst, opsB,
                                     mybir.ActivationFunctionType.Copy)
            else:
                nc.vector.tensor_copy(dst, opsB)
            if ch == TCH - 1:
                dma_eng = nc.scalar if (tail and ct % 2 == 1) else nc.sync
                dma_eng.dma_start(
                    out=y_d[b, ct * 128:(ct + 1) * 128, :],
                    in_=otw[ct][:])

        def em_outproj_b(i, idx, tail=False):
            opsB = ps.tile([128, 512], F32, tag="ps", name=f"psob{i}_{idx}")
            em_b_open(i, idx, opsB[:])
            em_b_close(i, idx, opsB[:], tail=tail)

        # ---- image-0 startup: minimal path to the first exp ------------
        # wq streams on the gpsimd queue while xb + wk stream on sync so
        # the first matmul's deps arrive in parallel; only pair-0 qkv and
        # v(g0) are emitted before the head loop -- the rest of image 0
        # rides the fill slots of its own attention
        state[0] = {"x": [], "xb": [], "q": {}, "k": {}, "v": {}, "r": None,
                    "oa": {}, "otw": {}}
        em_warm_pe()
        # the first four DMAs gate the first matmul: spread them over all
        # three DGE queues (sync HWDGE, ACT DGE, Pool SWDGE) so they land
        # in parallel instead of serializing on one queue
        for cp in range(CCP):
            wt = w8_p.tile([128, CCP, NH * DK], FP8, tag="wq",
                           name=f"wq{cp}", bufs=CCP)
            nc.scalar.dma_start(out=wt[:], in_=wq_d[cp])
            w_kind["q"].append(wt)
            xb = xb_p.tile([128, CCP, NTOK], FP8, tag="xbt", name=f"xb0_{cp}")
            (nc.sync if cp == 0 else nc.gpsimd).dma_start(
                out=xb[:], in_=xb_d[0, cp])
            state[0]["xb"].append(xb)
        em_weights("k")
        bqp = em_bqp()
        bkp, bv_bc, ebias = em_biases()
        em_weights("v")
        ident, identb = em_idents()
        em_wout()
        for ch in range(TCH):
            for kind in ("q", "k"):
                em_qkv_unit(0, 0, kind, ch,
                            eng=nc.scalar if kind == "k" else None)

        HEAD_ORDER = [1, 0, 3, 2, 5, 4, 7, 6]  # odd first: the odd head's
        # partition-shift DMA runs under the even head's attention, so the
        # pair tile is complete (outproj-ready) right when the pair ends.

        def fill_units0():
            # image 0's remaining projections, need-ordered: v groups ahead
            # of the AV that reads them, later qkv pairs ahead of their
            # heads.  The first slot is a no-op so v(g0) lands after the
            # first score matmul in the PE queue (it gates the first exp).
            yield lambda: None
            yield lambda: (em_v(0, 0), em_v(0, 1))
            for tt in range(2, NH):
                yield lambda tt=tt: em_v(0, tt)
            for m in range(1, NPAIR):
                for kind in ("q", "k"):
                    for ch in range(TCH):
                        yield (lambda m=m, kind=kind, ch=ch:
                               em_qkv_unit(0, m, kind, ch))
            yield lambda: em_xres(0)

        def fill_units(i):
            # fine-grained projection fill items for the ACT-bound j-loops
            if i < n_imgs:
                yield lambda: em_x(i)
                for m in range(NPAIR):
                    for kind in ("q", "k"):
                        for ch in range(TCH):
                            yield (lambda m=m, kind=kind, ch=ch:
                                   em_qkv_unit(i, m, kind, ch))
                for tt in range(NH):
                    yield lambda tt=tt: em_v(i, tt)
                yield lambda: em_xres(i)

        OUT_UNITS = CCH * TCH
        for i in range(n_imgs):
            fill = []
            if i == 0:
                fill += list(fill_units0())
            fill += list(fill_units(i + 1))
            if i - 1 >= 0:
                # space the B units so their matmuls don't burst between
                # consecutive score matmuls and stall the exp stream
                for p in range(OUT_UNITS):
                    fill += [lambda pidx=p, im=i - 1: em_outproj_b(im, pidx),
                             lambda: None]
            # A-half of this image's outproj, gated past the pair-2 flush
            # (head-7 j0, tick 49) and split in two half-emissions so no
            # burst exceeds ~2 matmuls between consecutive score matmuls
            a_units = []
            for p in range(OUT_UNITS):
                a_units += [
                    lambda pidx=p, im=i, spill=False: em_outproj_a(
                        im, pidx, 0, spill=spill),
                    lambda pidx=p, im=i, spill=False: em_outproj_a(
                        im, pidx, 1, spill=spill)]
            stride = max(1, (NH * NH) // max(1, len(fill) + OUT_UNITS))
            fi = 0
            tick = 0
            a_fi = 0
            for hp, h in enumerate(HEAD_ORDER):
                is_last = (i == n_imgs - 1 and h == HEAD_ORDER[-1])
                # offload only where DVE has real slack: the last image's
                # early heads (its fill is just the B-units of the previous
                # image); elsewhere the A/norm/fill copies crowd DVE and the
                # late pt stalls the in-order PE queue
                # hp5 (head 4) must NOT be offloaded: its delayed flush
                # would finalize pair-2's r tile after the outproj-A units
                # (gated at tick 50) read it -- an emission-order hazard
                sch = i == n_imgs - 1 and hp in (0, 1, 2, 3, 4)
                for _ in em_attn_head(i, h, last=is_last, sch=sch):
                    tick += 1
                    if fi < len(fill) and tick % stride == 0:
                        fill[fi]()
                        fi += 1
                    elif (fi >= len(fill) and tick >= 2 + NH * (NH - 2)
                          and a_fi < len(a_units)):
                        a_units[a_fi]()
                        a_fi += 1
            while fi < len(fill):
                fill[fi]()
                fi += 1
            if i == n_imgs - 1:
                # final image: emit the deferred AV+norm first so the
                # drain-critical chain leads the PE/DVE queues
                flush_pend()
            while a_fi < len(a_units):
                # spilled A-parts keep their copies off the DVE so the
                # final normalization chain owns it
                a_units[a_fi](spill=True)
                a_fi += 1
            if i - 2 in state:
                del state[i - 2]
        flush_pend()
        # drain: open six B groups up front (2 free ps slots + the freed
        # sps banks) so their identity matmuls run during the final
        # normalization; only the wo3*r3 matmuls wait for it
        li = n_imgs - 1
        wide = [ps_big.tile([128, NTOK], F32, tag="sps", name=f"psbw{g}")
                for g in range(2)]
        slots = {}
        for idx in range(6):
            if idx < 2:
                slots[idx] = ps.tile([128, 512], F32, tag="ps",
                                     name=f"psob{idx}")[:]
            else:
                g, half = divmod(idx - 2, 2)
                slots[idx] = wide[g][:, half * 512:(half + 1) * 512]
            em_b_open(li, idx, slots[idx])
        for idx in range(6):
            em_b_close(li, idx, slots[idx], tail=True)
        for idx in (6, 7):
            em_outproj_b(li, idx, tail=True)


def build_program(mode=MODE, repeat=1):
    nc = bacc.Bacc("TRN2", target_bir_lowering=False, debug=False)
    dt = nc.dram_tensor
    x_d = dt("x_loc", [B_LOC, C, NTOK], BF16, kind="ExternalInput").ap()
    xb_d = dt("xb_loc", [B_LOC, CCP, 128, CCP, NTOK], FP8,
              kind="ExternalInput").ap()
    wq_d = dt("wq", [CCP, 128, CCP, NH * DK], FP8, kind="ExternalInput").ap()
    wk_d = dt("wk", [CCP, 128, CCP, NH * DK], FP8, kind="ExternalInput").ap()
    wv_d = dt("wv", [CCP, 128, CCP, NH * DK], FP8, kind="ExternalInput").ap()
    wo_d = dt("wo", [NH * DK, C], BF16, kind="ExternalInput").ap()
    bqp_d = dt("bqp", [128, NPAIR], F32, kind="ExternalInput").ap()
    bkp_d = dt("bkp", [128, NPAIR], F32, kind="ExternalInput").ap()
    id_d = dt("ident", [128, 128], F32R, kind="ExternalInput").ap()
    idb_d = dt("identb", [128, 128], BF16, kind="ExternalInput").ap()
    bv_d = dt("bv", [1, NH * DK], F32, kind="ExternalInput").ap()
    y_d = dt("y", [B_LOC, C, NTOK], BF16, kind="ExternalOutput").ap()
    with tile.TileContext(nc) as tc:
        _emit(tc, x_d, xb_d, wq_d, wk_d, wv_d, wo_d, bqp_d, bkp_d, id_d,
              idb_d, bv_d, y_d, repeat=repeat)
    nc.compile()
    return nc


_NC_CACHE = {}


def _get_program(mode=MODE, repeat=1):
    key = (mode, repeat)
    if key not in _NC_CACHE:
        _NC_CACHE[key] = build_program(mode, repeat)
    return _NC_CACHE[key]


def host_prep(inputs):
    """Pre-gather weights per kind, scale x16, cast to fp8, shape biases."""
    f8 = ml_dtypes.float8_e4m3
    bf16 = ml_dtypes.bfloat16
    x = np.ascontiguousarray(np.asarray(inputs["x"], dtype=np.float32))
    B = x.shape[0]
    bo_early = np.asarray(inputs["b_out"], dtype=np.float32)
    x0 = x.reshape(B, C, NTOK)                   # projection input
    xs = x0 + bo_early[None, :, None]            # residual path: x + b_out
    wp = np.asarray(inputs["w_proj"], dtype=np.float32)
    bp = np.asarray(inputs["b_proj"], dtype=np.float32)
    wo = np.asarray(inputs["w_out"], dtype=np.float32)
    bo = np.asarray(inputs["b_out"], dtype=np.float32)

    w3 = wp.reshape(C, NH, 3, DK) * WSC         # [c, h, {q,k,v}, d] x16
    bp3 = bp.reshape(NH, 3, DK) * WSC

    def pack_w(kidx):
        # channel c = cp*256 + t*128 + p -> [cp, p, t, out] DoubleRow layout
        w = w3[:, :, kidx, :].reshape(CCP, CCP, 128, NH * DK)
        return np.ascontiguousarray(w.transpose(0, 2, 1, 3).astype(f8))

    common = {
        "wq": pack_w(0),
        "wk": pack_w(1),
        "wv": pack_w(2),
        "wo": np.ascontiguousarray(wo.astype(bf16)),
        # q/k/out biases as per-partition columns (pair / c-tile layout)
        "bqp": np.ascontiguousarray(
            bp3[:, 0, :].reshape(NPAIR, 128).T.astype(np.float32)),
        "bkp": np.ascontiguousarray(
            bp3[:, 1, :].reshape(NPAIR, 128).T.astype(np.float32)),
        "ident": np.ascontiguousarray(np.eye(128, dtype=np.float32)),
        "identb": np.ascontiguousarray(np.eye(128).astype(bf16)),
        "bv": np.ascontiguousarray(
            bp3[:, 2, :].reshape(1, NH * DK).astype(np.float32)),
    }
    xb8 = np.ascontiguousarray(
        x0.reshape(B, CCP, CCP, 128, NTOK).transpose(0, 1, 3, 2, 4)
        .astype(f8))
    return xs, xb8, common


def run(inputs, mode=MODE, trace=False, repeat=1):
    """Run on 8 cores; returns (y_full [16,512,32,32] f32, results)."""
    xs, xb8, common = host_prep(inputs)
    B = xs.shape[0]
    nc = _get_program(mode, repeat)
    in_maps = []
    for c in range(N_CORES):
        m = {"x_loc": np.ascontiguousarray(
                 xs[c * B_LOC:(c + 1) * B_LOC].astype(ml_dtypes.bfloat16)),
             "xb_loc": np.ascontiguousarray(xb8[c * B_LOC:(c + 1) * B_LOC])}
        m.update(common)
        in_maps.append(m)
    res = run_bass_kernel_spmd(nc, in_maps, core_ids=list(range(N_CORES)),
                               trace=trace)
    y = np.concatenate([res.results[c]["y"] for c in range(N_CORES)], axis=0)
    return y.astype(np.float32).reshape(B, C, 32, 32), res


def kernel(**inputs):
    y, _ = run(inputs)
    return y


if __name__ == "__main__":
    nc = build_program()
    print("program built + compiled OK")
